# revision 1
# baseline (speedup 1.0000x reference)
"""Trainium2 Bass kernel for nn_Microscope (scatter_memory), v2.

Data-parallel over batch: core c owns slab b=c (H=128, W=128, D=64).
Per-core pipeline:
  1. Extraction: host premultiplies locs by (chunk-local iota+1) in fp16;
     device runs 16x max8 -> flat indices per (row, 512-chunk).
  2. Per-w-bucket compaction (sparse_gather) into 24 batch-cols of 128.
  3. One dma_gather of host-interleaved [x|y|z|i] 256-elem fp16 lines;
     one-hot extract of the d element on DVE.
  4. Profiles: Act-engine square+exp in fp16; LH placed via local_scatter
     of amp-scaled 9-tap rows; LY/LZ placed unmasked (tails ~0, cropped
     by fp16 underflow + 9-tap-exact normalization).
  5. rhs_b = LY (x) LZ fp16 outer products (DVE 2x dup-trick / Pool split).
  6. Own-region psum: psum_t[h, 16w x 64d] accumulates fp16 matmuls from
     buckets t-1,t,t+1; direct PSUM->DRAM DMA, no crop-add pass.
"""
import sys
for _p in ('/opt/trn_rl_repo',):
    if _p not in sys.path:
        sys.path.insert(0, _p)
import math
import numpy as np

import concourse.bass as bass
import concourse.bacc as bacc
import concourse.mybir as mybir
import concourse.tile as tile

F32 = mybir.dt.float32
F16 = mybir.dt.float16
I32 = mybir.dt.int32
U32 = mybir.dt.uint32
I16 = mybir.dt.int16
AF = mybir.ActivationFunctionType
OP = mybir.AluOpType
X = mybir.AxisListType.X

H, W, D = 128, 128, 64
PSF, R_ = 9, 4
SINV = 1.0 / math.sqrt(4.5)     # scale s.t. Square(s*u) = u^2/4.5
NBUCK, NBPB = 8, 3
NB = NBUCK * NBPB               # 24 batch columns
WJ = 24                         # per-bucket w window (16 + 2*4 halo)
WK = 64                         # d window = full D (crop automatic)
NCHUNK = 16
IN_NAMES = ["locs", "vals4", "scb"]
N_CORES = 8
N_RHS_POOL = 9                  # rhs tiles computed on Pool engine (rest DVE)
USE_GSEM = True                 # explicit gather-completion sem (HW needs it)


def body_v2(tc, outs, ins, dbg=None):
    nc = tc.nc
    out_d = outs[0]
    locs_d, vals4_d, scb_d = ins

    def dump(name, t):
        if dbg is not None and name in dbg:
            nc.sync.dma_start(out=dbg[name], in_=t[:])
    with (tc.tile_pool(name="pool", bufs=1) as pool,
          tc.tile_pool(name="outp", bufs=2) as outp,
          tc.tile_pool(name="psump", bufs=2, space="PSUM") as psump,
          tc.tile_pool(name="dram", bufs=1, space="DRAM") as dpool):
        # ---- constant iotas ----
        io9 = pool.tile([128, 9], F32, name="io9", tag="io9")
        nc.gpsimd.iota(io9[:], pattern=[[1, 9]], base=-4, channel_multiplier=0,
                       allow_small_or_imprecise_dtypes=True)
        io24 = pool.tile([128, 24], F32, name="io24", tag="io24")
        nc.gpsimd.iota(io24[:], pattern=[[1, 24]], base=0, channel_multiplier=0,
                       allow_small_or_imprecise_dtypes=True)
        io64 = pool.tile([128, 64], F32, name="io64", tag="io64")
        nc.gpsimd.iota(io64[:], pattern=[[1, 64]], base=0, channel_multiplier=0,
                       allow_small_or_imprecise_dtypes=True)
        io64h = pool.tile([128, 64], F16, name="io64h", tag="io64h")
        nc.gpsimd.iota(io64h[:], pattern=[[1, 64]], base=0,
                       channel_multiplier=0,
                       allow_small_or_imprecise_dtypes=True)
        basei = pool.tile([128, 128], F32, name="basei", tag="basei")
        nc.gpsimd.iota(basei[:], pattern=[[512, NCHUNK], [0, 8]], base=-1,
                       channel_multiplier=8192,
                       allow_small_or_imprecise_dtypes=True)
        woff = pool.tile([128, NB], F32, name="woff", tag="woff")
        nc.gpsimd.iota(woff[:], pattern=[[16, NBUCK], [0, NBPB]], base=-4,
                       channel_multiplier=0, allow_small_or_imprecise_dtypes=True)
        ib216 = pool.tile([128, 216], F32, name="ib216", tag="ib216")
        nc.gpsimd.iota(ib216[:], pattern=[[0, 3], [128, 8], [0, 9]], base=0,
                       channel_multiplier=0, allow_small_or_imprecise_dtypes=True)

        # ---- E: extraction (locs premultiplied on host, fp16) ----
        locs_t = pool.tile([128, 8192], F16, name="locs_t", tag="w1")
        MX16 = pool.tile([128, 128], F16, name="MX16", tag="MX16")
        for h in range(4):
            sl = slice(h * 2048, (h + 1) * 2048)
            nc.sync.dma_start(out=locs_t[:, sl], in_=locs_d[:, sl])
            for c in range(h * 4, h * 4 + 4):
                nc.vector.max(MX16[:, c * 8:(c + 1) * 8],
                              locs_t[:, c * 512:(c + 1) * 512])
        MX = pool.tile([128, 128], F32, name="MX", tag="MX")
        nc.vector.tensor_copy(MX[:], MX16[:])
        dump('d_mx16', MX16)
        vld0 = pool.tile([128, 128], F32, name="vld0", tag="vld0")
        nc.vector.tensor_scalar(out=vld0[:], in0=MX[:], scalar1=0.5,
                                scalar2=None, op0=OP.is_gt)
        nc.vector.tensor_tensor(out=MX[:], in0=MX[:], in1=basei[:], op=OP.add)
        nc.vector.tensor_tensor(out=MX[:], in0=MX[:], in1=vld0[:], op=OP.mult)
        nc.vector.scalar_tensor_tensor(out=MX[:], in0=vld0[:], scalar=1.0,
                                       in1=MX[:], op0=OP.subtract, op1=OP.add)
        # MX: flat idx (>=0) or -1

        # bounce 1: (128h,128slot) -> SG_IN [16, 8 buckets * 128]
        dump('d_mx', MX)
        flat1 = dpool.tile([1, 16384], F32, name="flat1", tag="flat1")
        f1w = flat1[:].rearrange("a (p c) -> (a p) c", p=128)
        nc.sync.dma_start(out=f1w, in_=MX[:])
        f1r = flat1[:].rearrange("a (h16 p2 t c) -> (a p2) t h16 c",
                                 h16=8, p2=16, t=8, c=16)
        SG_IN = pool.tile([16, 1024], F32, name="SG_IN", tag="SG_IN")
        sgin_v = SG_IN[:].rearrange("p (t h16 c) -> p t h16 c", t=8, h16=8, c=16)
        nc.sync.dma_start(out=sgin_v, in_=f1r)

        SG_OUT = pool.tile([16, 192], F32, name="SG_OUT", tag="SG_OUT")
        NF = pool.tile([1, 8], U32, name="NF", tag="NF")
        for t in range(NBUCK):
            nc.gpsimd.sparse_gather(SG_OUT[:, t * 24:(t + 1) * 24],
                                    SG_IN[:, t * 128:(t + 1) * 128],
                                    num_found=NF[:, t:t + 1])
        NF16 = pool.tile([16, 8], U32, name="NF16", tag="NF16")
        nc.gpsimd.partition_broadcast(NF16[:], NF[:], channels=16)
        NF16f = pool.tile([16, 8], F32, name="NF16f", tag="NF16f")
        nc.vector.tensor_copy(NF16f[:], NF16[:])
        SP = pool.tile([16, 192], F32, name="SP", tag="SP")
        nc.gpsimd.iota(SP[:], pattern=[[0, 8], [16, 24]], base=0,
                       channel_multiplier=1, allow_small_or_imprecise_dtypes=True)
        SV = pool.tile([16, 192], F32, name="SV", tag="SV")
        nf_b = NF16f[:].rearrange("p t -> p t ()").broadcast_to([16, 8, 24])
        nc.vector.tensor_tensor(out=SV[:].rearrange("p (t f) -> p t f", f=24),
                                in0=SP[:].rearrange("p (t f) -> p t f", f=24),
                                in1=nf_b, op=OP.is_lt)
        nc.vector.tensor_tensor(out=SG_OUT[:], in0=SG_OUT[:], in1=SV[:],
                                op=OP.mult)
        nc.vector.scalar_tensor_tensor(out=SG_OUT[:], in0=SV[:], scalar=1.0,
                                       in1=SG_OUT[:], op0=OP.subtract, op1=OP.add)
        nc.vector.tensor_scalar(out=SG_OUT[:], in0=SG_OUT[:], scalar1=0.0,
                                scalar2=None, op0=OP.max)
        # idx16 for dma_gather: hw idx = flat >> 6
        dump('d_sgout', SG_OUT)
        dump('d_sv', SV)
        LI = pool.tile([16, 192], I32, name="LI", tag="LI")
        nc.vector.tensor_copy(LI[:], SG_OUT[:])
        HWI = pool.tile([16, 192], I32, name="HWI", tag="HWI")
        nc.vector.tensor_scalar(out=HWI[:], in0=LI[:], scalar1=6, scalar2=None,
                                op0=OP.arith_shift_right)
        IDX16s = pool.tile([16, 192], I16, name="IDX16s", tag="IDX16s")
        nc.vector.tensor_copy(IDX16s[:], HWI[:])
        IDX = pool.tile([128, 192], I16, name="IDX", tag="IDX")
        for g in range(8):
            nc.scalar.dma_start(out=IDX[g * 16:(g + 1) * 16, :], in_=IDX16s[:])

        # bounce 2: wrapped [16,192] -> batch layout (128, 24)
        dump('d_idx', IDX)
        flat2 = dpool.tile([1, 6144], F32, name="flat2", tag="flat2")
        f2w_l = flat2[:, 0:3072].rearrange("a (t f p2) -> (a p2) t f",
                                           t=8, f=24, p2=16)
        nc.sync.dma_start(out=f2w_l, in_=SG_OUT[:].rearrange(
            "p (t f) -> p t f", t=8, f=24))
        f2w_v = flat2[:, 3072:6144].rearrange("a (t f p2) -> (a p2) t f",
                                              t=8, f=24, p2=16)
        nc.sync.dma_start(out=f2w_v, in_=SV[:].rearrange(
            "p (t f) -> p t f", t=8, f=24))
        LT = pool.tile([128, NB], F32, name="LT", tag="LT")
        nc.sync.dma_start(out=LT[:], in_=flat2[:, 0:3072].rearrange(
            "a (t j p) -> (a p) (t j)", t=8, j=3, p=128))
        VT = pool.tile([128, NB], F32, name="VT", tag="VT")
        nc.sync.dma_start(out=VT[:], in_=flat2[:, 3072:6144].rearrange(
            "a (t j p) -> (a p) (t j)", t=8, j=3, p=128))

        # ---- value gather: one 3072-idx gather of interleaved fp16 lines ----
        dump('d_lt', LT)
        dump('d_vt', VT)
        GV4 = pool.tile([128, NB * 256], F16, name="GV4", tag="GV4")
        gsem = (nc.alloc_semaphore(f"gsem_{nc.next_id()}")
                if USE_GSEM else None)
        for cg in range(3):
            g = nc.gpsimd.dma_gather(
                out_ap=GV4[:, cg * 2048:(cg + 1) * 2048].rearrange(
                    "p (c k) -> p c k", k=256),
                in_ap=vals4_d[:].rearrange("p (w k) -> (p w) k", k=256),
                idxs_ap=IDX[:, cg * 64:(cg + 1) * 64],
                num_idxs=1024, num_idxs_reg=1024, elem_size=256)
            if gsem is not None:
                g.then_inc(gsem, 16)
        if gsem is not None:
            nc.vector.wait_ge(gsem, 48)

        # ---- decode positions ----
        dump('d_gv4', GV4)
        LTi = pool.tile([128, NB], I32, name="LTi", tag="LTi")
        nc.vector.tensor_copy(LTi[:], LT[:])
        phi = pool.tile([128, NB], I32, name="phi", tag="phi")
        nc.vector.tensor_scalar(out=phi[:], in0=LTi[:], scalar1=13, scalar2=None,
                                op0=OP.arith_shift_right)
        pwi = pool.tile([128, NB], I32, name="pwi", tag="pwi")
        nc.vector.tensor_scalar(out=pwi[:], in0=LTi[:], scalar1=6, scalar2=127,
                                op0=OP.arith_shift_right, op1=OP.bitwise_and)
        pdi = pool.tile([128, NB], I32, name="pdi", tag="pdi")
        nc.vector.tensor_scalar(out=pdi[:], in0=LTi[:], scalar1=63, scalar2=None,
                                op0=OP.bitwise_and)
        ph = pool.tile([128, NB], F32, name="ph_t", tag="ph_t")
        nc.vector.tensor_copy(ph[:], phi[:])
        pw = pool.tile([128, NB], F32, name="pw_t", tag="pw_t")
        nc.vector.tensor_copy(pw[:], pwi[:])
        pd = pool.tile([128, NB], F32, name="pd_t", tag="pd_t")
        nc.vector.tensor_copy(pd[:], pdi[:])

        # ---- extract 4 values per slot via one-hot over d (all fp16 2x) ----
        pdh2 = pool.tile([128, NB * 2], F16, name="pdh2", tag="pdh2")
        pd3 = pd[:].rearrange("p c -> p c ()").broadcast_to([128, NB, 2])
        nc.vector.tensor_copy(pdh2[:].rearrange("p (c u) -> p c u", u=2), pd3)
        OH = pool.tile([128, NB * 64], F16, name="OH", tag="OH")
        io64_b = io64h[:].rearrange("p (k2 u) -> p () k2 u", u=2).broadcast_to(
            [128, NB, 32, 2])
        pdh_b = pdh2[:].rearrange("p (c u) -> p c () u", u=2).broadcast_to(
            [128, NB, 32, 2])
        nc.vector.tensor_tensor(
            out=OH[:].rearrange("p (c k2 u) -> p c k2 u", k2=32, u=2),
            in0=io64_b, in1=pdh_b, op=OP.is_equal)
        GVm = pool.tile([128, NB * 256], F16, name="GVm", tag="GVm")
        oh_b = OH[:].rearrange("p (c k) -> p c () k", k=64).broadcast_to(
            [128, NB, 4, 64])
        nc.vector.tensor_tensor(
            out=GVm[:].rearrange("p (c s k) -> p c s k", s=4, k=64),
            in0=GV4[:].rearrange("p (c s k) -> p c s k", s=4, k=64),
            in1=oh_b, op=OP.mult)
        gvm4 = GVm[:].rearrange("p (c s k) -> p c s k", s=4, k=64)
        n = 32
        while n >= 1:
            nc.vector.tensor_tensor(out=gvm4[:, :, :, 0:n],
                                    in0=gvm4[:, :, :, 0:n],
                                    in1=gvm4[:, :, :, n:2 * n], op=OP.add)
            n //= 2
        V96 = pool.tile([128, NB * 4], F16, name="V96", tag="V96")
        nc.vector.tensor_copy(V96[:].rearrange("p (c s) -> p c s", s=4),
                              gvm4[:, :, :, 0:1].rearrange("p c s k -> p c (s k)"
                                                           ).rearrange(
                                  "p c s -> p c s"))
        V4 = pool.tile([128, NB * 4], F32, name="V4", tag="V4")
        dump('d_v96', V96)
        nc.vector.tensor_copy(V4[:], V96[:])
        v3 = V4[:].rearrange("p (c s) -> p c s", s=4)
        VX, VY, VZ, VI = (v3[:, :, 0], v3[:, :, 1], v3[:, :, 2], v3[:, :, 3])

        # ---- 9-tap exact per-axis sums (normalization) + compact X profile ----
        AX = pool.tile([128, 216], F32, name="AX", tag="AX")
        AY = pool.tile([128, 216], F32, name="AY", tag="AY")
        AZ = pool.tile([128, 216], F32, name="AZ", tag="AZ")
        io9_b = io9[:].rearrange("p t -> p () t").broadcast_to([128, NB, 9])
        for A, V in ((AX, VX), (AY, VY), (AZ, VZ)):
            v_b = V.rearrange("p c -> p c ()").broadcast_to([128, NB, 9])
            nc.vector.tensor_tensor(out=A[:].rearrange("p (c t) -> p c t", t=9),
                                    in0=io9_b, in1=v_b, op=OP.subtract)
            nc.scalar.activation(A[:], A[:], AF.Square, scale=SINV)
            nc.scalar.activation(A[:], A[:], AF.Exp, scale=-1.0)
        s_x = pool.tile([128, NB], F32, name="sx", tag="sx")
        s_y = pool.tile([128, NB], F32, name="sy", tag="sy")
        s_z = pool.tile([128, NB], F32, name="sz", tag="sz")
        for s, A in ((s_x, AX), (s_y, AY), (s_z, AZ)):
            nc.vector.tensor_reduce(out=s[:], axis=X,
                                    in_=A[:].rearrange("p (c t) -> p c t", t=9),
                                    op=OP.add)
        nc.vector.tensor_tensor(out=s_x[:], in0=s_x[:], in1=s_y[:], op=OP.mult)
        nc.vector.tensor_tensor(out=s_x[:], in0=s_x[:], in1=s_z[:], op=OP.mult)
        amp = pool.tile([128, NB], F32, name="amp", tag="amp")
        nc.vector.reciprocal(amp[:], s_x[:])
        nc.vector.tensor_tensor(out=amp[:], in0=amp[:], in1=VI[:], op=OP.mult)
        scb_t = pool.tile([128, 1], F32, name="scb_t", tag="scb_t")
        nc.scalar.dma_start(out=scb_t[:], in_=scb_d[:])
        sc_b = scb_t[:].broadcast_to([128, NB])
        nc.vector.tensor_tensor(out=amp[:], in0=amp[:], in1=sc_b, op=OP.mult)
        nc.vector.tensor_tensor(out=amp[:], in0=amp[:], in1=VT[:], op=OP.mult)

        # ---- LH: amp-scaled compact 9-tap rows placed via local_scatter ----
        dump('d_amp', amp)
        D9 = pool.tile([128, 216], F16, name="D9", tag="D9")
        amp_b = amp[:].rearrange("p c -> p c ()").broadcast_to([128, NB, 9])
        nc.vector.tensor_tensor(out=D9[:].rearrange("p (c t) -> p c t", t=9),
                                in0=AX[:].rearrange("p (c t) -> p c t", t=9),
                                in1=amp_b, op=OP.mult)
        posl = pool.tile([128, 216], F32, name="posl", tag="posl")
        ph_b = ph[:].rearrange("p c -> p c ()").broadcast_to([128, NB, 9])
        nc.vector.tensor_tensor(out=posl[:].rearrange("p (c t) -> p c t", t=9),
                                in0=io9_b, in1=ph_b, op=OP.add)
        mbad = pool.tile([128, 216], F32, name="mbad", tag="mbad")
        nc.vector.tensor_scalar(out=mbad[:], in0=posl[:], scalar1=0.0,
                                scalar2=None, op0=OP.is_lt)
        mb2 = pool.tile([128, 216], F32, name="mb2", tag="mb2")
        nc.vector.tensor_scalar(out=mb2[:], in0=posl[:], scalar1=127.0,
                                scalar2=None, op0=OP.is_gt)
        nc.vector.tensor_tensor(out=mbad[:], in0=mbad[:], in1=mb2[:], op=OP.add)
        nc.vector.tensor_tensor(out=posl[:], in0=posl[:], in1=ib216[:],
                                op=OP.add)
        nc.vector.scalar_tensor_tensor(out=posl[:], in0=mbad[:], scalar=-9999.0,
                                       in1=posl[:], op0=OP.mult, op1=OP.add)
        I9 = pool.tile([128, 216], I16, name="I9", tag="I9")
        nc.vector.tensor_copy(I9[:], posl[:])
        LH = pool.tile([128, NB * 128], F16, name="LH", tag="LH")
        for g in range(3):
            nc.gpsimd.local_scatter(
                out_ap=LH[:, g * 1024:(g + 1) * 1024],
                data_ap=D9[:, g * 72:(g + 1) * 72],
                idxs_ap=I9[:, g * 72:(g + 1) * 72],
                channels=128, num_elems=1024, num_idxs=72)

        # ---- LY (24-wide placed, dup layout) and LZ (64-wide placed) ----
        dump('d_lh', LH)
        dump('d_d9', D9)
        dump('d_i9', I9)
        argY = pool.tile([128, NB * WJ], F32, name="argY", tag="argY")
        poY = pool.tile([128, NB], F32, name="poY", tag="poY")
        nc.vector.tensor_tensor(out=poY[:], in0=pw[:], in1=woff[:],
                                op=OP.subtract)
        nc.vector.tensor_tensor(out=poY[:], in0=poY[:], in1=VY[:], op=OP.add)
        io24_b = io24[:].rearrange("p j -> p () j").broadcast_to([128, NB, WJ])
        poY_b = poY[:].rearrange("p c -> p c ()").broadcast_to([128, NB, WJ])
        nc.vector.tensor_tensor(out=argY[:].rearrange("p (c j) -> p c j", j=WJ),
                                in0=io24_b, in1=poY_b, op=OP.subtract)
        nc.scalar.activation(argY[:], argY[:], AF.Square, scale=SINV)
        LY2 = pool.tile([128, NB * WJ * 2], F16, name="LY2", tag="LY2")
        ly2v = LY2[:].rearrange("p (cj u) -> p cj u", u=2)
        nc.scalar.activation(ly2v[:, :, 0], argY[:], AF.Exp, scale=-1.0)
        nc.scalar.activation(ly2v[:, :, 1], argY[:], AF.Exp, scale=-1.0)

        argZ = pool.tile([128, NB * WK], F32, name="argZ", tag="argZ")
        poZ = pool.tile([128, NB], F32, name="poZ", tag="poZ")
        nc.vector.tensor_tensor(out=poZ[:], in0=pd[:], in1=VZ[:], op=OP.add)
        io64_bb = io64[:].rearrange("p k -> p () k").broadcast_to([128, NB, WK])
        poZ_b = poZ[:].rearrange("p c -> p c ()").broadcast_to([128, NB, WK])
        nc.vector.tensor_tensor(out=argZ[:].rearrange("p (c k) -> p c k", k=WK),
                                in0=io64_bb, in1=poZ_b, op=OP.subtract)
        nc.scalar.activation(argZ[:], argZ[:], AF.Square, scale=SINV)
        LZ = pool.tile([128, NB * WK], F16, name="LZ", tag="LZ")
        nc.scalar.activation(LZ[:], argZ[:], AF.Exp, scale=-1.0)

        # ---- rhs outer products: rhs_b[p, (j,k)] = LY[p,j] * LZ[p,k] ----
        dump('d_ly2', LY2)
        dump('d_lz', LZ)
        RHS = pool.tile([128, NB * WJ * WK], F16, name="RHS", tag="RHS")
        n_pool = 0
        for b in range(NB):
            rhs3 = RHS[:, b * WJ * WK:(b + 1) * WJ * WK].rearrange(
                "p (j k) -> p j k", k=WK)
            lz_b = LZ[:, b * WK:(b + 1) * WK].rearrange(
                "p k -> p () k").broadcast_to([128, WJ, WK])
            if n_pool < N_RHS_POOL and (b % 3 == 1):
                ly_b = LY2[:, b * WJ * 2:(b + 1) * WJ * 2].rearrange(
                    "p (j u) -> p j u", u=2)[:, :, 0:1].broadcast_to(
                    [128, WJ, WK])
                nc.gpsimd.tensor_tensor(out=rhs3, in0=ly_b, in1=lz_b,
                                        op=OP.mult)
                n_pool += 1
            else:
                ly_d = LY2[:, b * WJ * 2:(b + 1) * WJ * 2].rearrange(
                    "p (j u) -> p j () u", u=2).broadcast_to([128, WJ, 32, 2])
                rhs4 = RHS[:, b * WJ * WK:(b + 1) * WJ * WK].rearrange(
                    "p (j k2 u) -> p j k2 u", k2=32, u=2)
                lz_d = LZ[:, b * WK:(b + 1) * WK].rearrange(
                    "p (k2 u) -> p () k2 u", u=2).broadcast_to([128, WJ, 32, 2])
                nc.vector.tensor_tensor(out=rhs4, in0=ly_d, in1=lz_d,
                                        op=OP.mult)

        # ---- matmuls: own-region psum per bucket, direct DMA out ----
        dump('d_rhs', RHS)
        # psum_t covers w in [16t, 16t+16) x d, 4 quarters of 256 cols.
        # quarter q: psum cols [256q, 256q+256).
        #   own bucket t: rhs cols [256+256q, 512+256q)
        #   left (t-1), q==0 only: rhs cols [1280, 1536)
        #   right (t+1), q==3 only: rhs cols [0, 256)
        for t in range(NBUCK):
            ps = psump.tile([128, 1024], F32, name=f"ps{t}", tag="ps")
            for bank in range(2):
                # full-bank own-bucket matmuls first (start resets the bank),
                # then the neighbor's 256-wide partial-bank accumulates.
                mms = [(t * NBPB + j, 256 + 512 * bank, 512, 512 * bank)
                       for j in range(NBPB)]
                if bank == 0 and t > 0:
                    mms += [((t - 1) * NBPB + j, 1280, 256, 0)
                            for j in range(NBPB)]
                if bank == 1 and t < NBUCK - 1:
                    mms += [((t + 1) * NBPB + j, 0, 256, 768)
                            for j in range(NBPB)]
                for i, (b, c0, wd, p0) in enumerate(mms):
                    nc.tensor.matmul(
                        ps[:, p0:p0 + wd],
                        lhsT=LH[:, b * 128:(b + 1) * 128],
                        rhs=RHS[:, b * WJ * WK + c0:b * WJ * WK + c0 + wd],
                        start=(i == 0), stop=(i == len(mms) - 1),
                        skip_group_check=True)
            ot = outp.tile([128, 1024], F32, name=f"ot{t}", tag="ot")
            nc.scalar.copy(out=ot[:], in_=ps[:])
            nc.sync.dma_start(out=out_d[:, t * 1024:(t + 1) * 1024], in_=ot[:])


DBG_SPECS = {
    "d_mx16": ([128, 128], F16), "d_mx": ([128, 128], F32),
    "d_sgout": ([16, 192], F32), "d_sv": ([16, 192], F32),
    "d_idx": ([128, 192], I16), "d_lt": ([128, 24], F32),
    "d_vt": ([128, 24], F32), "d_gv4": ([128, 6144], F16),
    "d_v96": ([128, 96], F16), "d_amp": ([128, 24], F32),
    "d_lh": ([128, 3072], F16), "d_d9": ([128, 216], F16),
    "d_i9": ([128, 216], I16), "d_ly2": ([128, 1152], F16),
    "d_lz": ([128, 1536], F16), "d_rhs": ([128, 36864], F16),
}


def build_nc(repeats=1, debug=False):
    nc = bacc.Bacc("TRN2", target_bir_lowering=False, debug=False,
                   num_devices=N_CORES)
    ins = []
    for nm in IN_NAMES:
        if nm == "locs":
            t = nc.dram_tensor(nm, [128, 8192], F16, kind="ExternalInput")
        elif nm == "vals4":
            t = nc.dram_tensor(nm, [128, 32768], F16, kind="ExternalInput")
        else:
            t = nc.dram_tensor(nm, [128, 1], F32, kind="ExternalInput")
        ins.append(t.ap())
    out_d = nc.dram_tensor("out", [128, W * D], F32, kind="ExternalOutput").ap()
    dbg = None
    if debug:
        names = (debug if isinstance(debug, (list, tuple, set))
                 else list(DBG_SPECS))
        dbg = {nm: nc.dram_tensor(nm, *DBG_SPECS[nm], kind="ExternalOutput").ap()
               for nm in names}
    with tile.TileContext(nc) as tc:
        for _rep in range(repeats):
            body_v2(tc, [out_d], ins, dbg=dbg if _rep == 0 else None)
    nc.compile()
    return nc


class _SpmdRunner:
    def __init__(self, nc, n_cores=N_CORES):
        import jax
        import jax.numpy as jnp
        from jax.sharding import Mesh, PartitionSpec
        from jax.experimental.shard_map import shard_map
        from concourse import bass2jax
        from concourse.bass2jax import _bass_exec_p, partition_id_tensor
        bass2jax.install_neuronx_cc_hook()
        self.jax, self.jnp = jax, jnp
        self.n_cores = n_cores
        in_names, out_names, out_avals, zero_outs = [], [], [], []
        pname = nc.partition_id_tensor.name if nc.partition_id_tensor else None
        for alloc in nc.m.functions[0].allocations:
            if not isinstance(alloc, mybir.MemoryLocationSet):
                continue
            name = alloc.memorylocations[0].name
            if alloc.kind == "ExternalInput":
                if name != pname:
                    in_names.append(name)
            elif alloc.kind == "ExternalOutput":
                shape = tuple(alloc.tensor_shape)
                dtype = mybir.dt.np(alloc.dtype)
                out_names.append(name)
                out_avals.append(jax.core.ShapedArray(shape, dtype))
                zero_outs.append(np.zeros(shape, dtype))
        self.in_names, self.out_names = in_names, out_names
        self.out_avals, self.zero_outs = out_avals, zero_outs
        n_params, n_outs = len(in_names), len(out_avals)
        all_in = in_names + out_names + ([pname] if pname else [])

        def _fn(*args):
            operands = list(args)
            if pname is not None:
                operands.append(partition_id_tensor())
            return tuple(_bass_exec_p.bind(
                *operands, out_avals=tuple(out_avals), in_names=tuple(all_in),
                out_names=tuple(out_names), lowering_input_output_aliases=(),
                sim_require_finite=True, sim_require_nnan=True, nc=nc))

        devices = jax.devices()[:n_cores]
        mesh = Mesh(np.asarray(devices), ("core",))
        specs = (PartitionSpec("core"),)
        self.sharded = jax.jit(
            shard_map(_fn, mesh=mesh, in_specs=specs * (n_params + n_outs),
                      out_specs=specs * n_outs),
            donate_argnums=tuple(range(n_params, n_params + n_outs)),
            keep_unused=True)

    def run(self, in_maps):
        concat = [np.concatenate([np.asarray(m[n]) for m in in_maps], axis=0)
                  for n in self.in_names]
        zeros = [self.jnp.zeros((self.n_cores * z.shape[0], *z.shape[1:]),
                                z.dtype) for z in self.zero_outs]
        outs = self.sharded(*concat, *zeros)
        self.jax.block_until_ready(outs)
        return [
            {n: np.asarray(outs[i]).reshape(self.n_cores,
                                            *self.out_avals[i].shape)[c]
             for i, n in enumerate(self.out_names)}
            for c in range(self.n_cores)]


_RUNNER_CACHE = {}


def _get_runner(repeats=1):
    if repeats not in _RUNNER_CACHE:
        _RUNNER_CACHE[repeats] = _SpmdRunner(build_nc(repeats))
    return _RUNNER_CACHE[repeats]


def _make_in_maps(locs_3d, x_os_3d, y_os_3d, z_os_3d, ints_3d, scale):
    sc = float(np.asarray(scale).reshape(-1)[0])
    scb = np.full((128, 1), 1000.0 * sc, np.float32)
    iota512 = np.tile(np.arange(512, dtype=np.float32) + 1.0, 16)
    in_maps = []
    for c in range(N_CORES):
        locs = np.asarray(locs_3d)[c, 0].reshape(128, 8192)
        locs_pre = (locs * iota512).astype(np.float16)
        sl = [np.asarray(t)[c, 0].reshape(128, 128, 64)
              for t in (x_os_3d, y_os_3d, z_os_3d, ints_3d)]
        vals4 = np.stack(sl, axis=2).astype(np.float16).reshape(128, 32768)
        in_maps.append({"locs": locs_pre, "vals4": np.ascontiguousarray(vals4),
                        "scb": scb})
    return in_maps


def kernel(locs_3d, x_os_3d, y_os_3d, z_os_3d, ints_3d, scale):
    runner = _get_runner()
    in_maps = _make_in_maps(locs_3d, x_os_3d, y_os_3d, z_os_3d, ints_3d, scale)
    res = runner.run(in_maps)
    out = np.stack([res[c]["out"].reshape(H, W, D) for c in range(N_CORES)])
    return out[:, None].astype(np.float32)



# revision 45
# speedup vs baseline: 1.2951x; 1.2951x over previous
"""Trainium2 Bass kernel for nn_Microscope (scatter_memory), v3.

Data-parallel over batch: core c owns slab b=c (H=128, W=128, D=64).
Pipelined over 4 groups of 2 w-buckets (16 w each):
  load locs piece -> max8 per 512-chunk (DVE) -> PE-transpose ->
  DRAM bounce (lane-contiguous) -> sparse_gather per bucket (Pool) ->
  merged SG|SV bounce -> PE-matmul idx replication -> dma_gather of
  32-site [x|y|z|i] lines -> one-hot-32 decode -> fused profiles ->
  RHS outer products (DVE/Pool split) -> per-bucket matmuls -> fp16
  copies -> out DMA.  Host converts fp16 output to f32.
"""
import sys
for _p in ('/opt/trn_rl_repo',):
    if _p not in sys.path:
        sys.path.insert(0, _p)
import math
import numpy as np

import concourse.bass as bass
import concourse.bacc as bacc
import concourse.mybir as mybir
import concourse.tile as tile

F32 = mybir.dt.float32
F16 = mybir.dt.float16
I32 = mybir.dt.int32
U32 = mybir.dt.uint32
I16 = mybir.dt.int16
AF = mybir.ActivationFunctionType
OP = mybir.AluOpType
X = mybir.AxisListType.X

H, W, D = 128, 128, 64
PSF, R_ = 9, 4
SINV = 1.0 / math.sqrt(4.5)     # scale s.t. (s*u)^2 = u^2/4.5
NBUCK, NBPB = 8, 3
NB = NBUCK * NBPB               # 24 batch columns
WJ = 24                         # per-bucket w window (16 + 2*4 halo)
WK = 64                         # d window = full D
NG, TPG, CPG = 4, 2, 6          # groups, buckets/group, cols/group
IN_NAMES = ["locs", "vals4", "scb"]
N_CORES = 8
NWARM = 8                       # PE warmup matmuls before real stream
# engine per RHS tile (g*6+i): mostly DVE (2x fp16), some Pool
RHS_POOL_MOD = 3                # i%3==2 -> Pool
TAIL_DVE = (0, 1)               # groups whose decode/misc go on DVE
COPY_ENG = ['act', 'act', 'act', 'act', 'dve', 'act', 'act', 'dve']


def body_v3(tc, outs, ins, dbg=None):
    nc = tc.nc
    out_d = outs[0]
    locs_d, vals4_d, scb_d = ins
    with (tc.tile_pool(name="pool", bufs=1) as pool,
          tc.tile_pool(name="outp", bufs=2) as outp,
          tc.tile_pool(name="psumP", bufs=2, space="PSUM") as psumP,
          tc.tile_pool(name="psumS", bufs=1, space="PSUM") as psumS,
          tc.tile_pool(name="dram", bufs=1, space="DRAM") as dpool):
        # ================= constants =================
        io9 = pool.tile([128, 9], F32, name="io9", tag="io9")
        nc.gpsimd.iota(io9[:], pattern=[[1, 9]], base=-4, channel_multiplier=0,
                       allow_small_or_imprecise_dtypes=True)
        io24 = pool.tile([128, 24], F32, name="io24", tag="io24")
        nc.gpsimd.iota(io24[:], pattern=[[1, 24]], base=0, channel_multiplier=0,
                       allow_small_or_imprecise_dtypes=True)
        io64 = pool.tile([128, 64], F32, name="io64", tag="io64")
        nc.gpsimd.iota(io64[:], pattern=[[1, 64]], base=0, channel_multiplier=0,
                       allow_small_or_imprecise_dtypes=True)
        io32h = pool.tile([128, 32], F16, name="io32h", tag="io32h")
        nc.gpsimd.iota(io32h[:], pattern=[[1, 32]], base=0,
                       channel_multiplier=0,
                       allow_small_or_imprecise_dtypes=True)
        basei = pool.tile([128, 128], F32, name="basei", tag="basei")
        nc.gpsimd.iota(basei[:], pattern=[[512, 16], [0, 8]], base=-1,
                       channel_multiplier=8192,
                       allow_small_or_imprecise_dtypes=True)
        woff = pool.tile([128, NB], F32, name="woff", tag="woff")
        nc.gpsimd.iota(woff[:], pattern=[[16, NBUCK], [0, NBPB]], base=-4,
                       channel_multiplier=0, allow_small_or_imprecise_dtypes=True)
        ib54 = pool.tile([128, 54], F32, name="ib54", tag="ib54")
        nc.gpsimd.iota(ib54[:], pattern=[[128, 6], [0, 9]], base=0,
                       channel_multiplier=0, allow_small_or_imprecise_dtypes=True)
        iden = pool.tile([128, 128], F32, name="iden", tag="iden")
        nc.gpsimd.iota(iden[:], pattern=[[1, 128]], base=0,
                       channel_multiplier=-1,
                       allow_small_or_imprecise_dtypes=True)
        nc.gpsimd.tensor_scalar(out=iden[:], in0=iden[:], scalar1=0.0,
                                scalar2=None, op0=OP.is_equal)
        # REP[k, p] = (p % 16 == k), for idx replication via PE
        repi = pool.tile([16, 128], I32, name="repi", tag="repi")
        nc.gpsimd.iota(repi[:], pattern=[[1, 128]], base=0,
                       channel_multiplier=-1,
                       allow_small_or_imprecise_dtypes=True)
        nc.vector.tensor_scalar(out=repi[:], in0=repi[:], scalar1=15,
                                scalar2=None, op0=OP.bitwise_and)
        nc.vector.tensor_scalar(out=repi[:], in0=repi[:], scalar1=0,
                                scalar2=None, op0=OP.is_equal)
        REP = pool.tile([16, 128], F32, name="REP", tag="REP")
        nc.gpsimd.tensor_copy(REP[:], repi[:])
        # per-slot rank iota for validity masks: SPc[p2, (tl, f24)] = f24*16+p2
        SPc = pool.tile([16, 48], F32, name="SPc", tag="SPc")
        nc.gpsimd.iota(SPc[:], pattern=[[0, 2], [16, 24]], base=0,
                       channel_multiplier=1, allow_small_or_imprecise_dtypes=True)
        scb_t = pool.tile([128, 1], F32, name="scb_t", tag="scb_t")
        nc.scalar.dma_start(out=scb_t[:], in_=scb_d[:])
        wrm = pool.tile([128, 512], F16, name="wrm", tag="wrm")
        nc.gpsimd.memset(wrm[:], 0.0)

        # ================= global tiles =================
        locs_t = pool.tile([128, 8192], F16, name="locs_t", tag="locs_t")
        MX16 = pool.tile([128, 128], F16, name="MX16", tag="MX16")

        G = [dict() for _ in range(NG)]   # per-group tiles

        def t_eng(g):
            return nc.vector if g in TAIL_DVE else nc.gpsimd

        # ---------------- stages ----------------
        def stage_load(g):
            sl = slice(g * 2048, (g + 1) * 2048)
            eng = nc.sync if g % 2 == 0 else nc.scalar
            eng.dma_start(out=locs_t[:, sl], in_=locs_d[:, sl])

        def stage_max(g):
            for c in range(g * 4, g * 4 + 4):
                nc.vector.max(MX16[:, c * 8:(c + 1) * 8],
                              locs_t[:, c * 512:(c + 1) * 512])

        def stage_mxpost(g):
            s = G[g]
            sl = slice(g * 32, (g + 1) * 32)
            MXg = pool.tile([128, 32], F32, name=f"MX{g}", tag=f"MX{g}")
            nc.gpsimd.tensor_copy(MXg[:], MX16[:, sl])
            vld = pool.tile([128, 32], F32, name=f"vld{g}", tag=f"vld{g}")
            nc.gpsimd.tensor_scalar(out=vld[:], in0=MXg[:], scalar1=0.5,
                                    scalar2=None, op0=OP.is_gt)
            nc.gpsimd.tensor_tensor(out=MXg[:], in0=MXg[:], in1=basei[:, sl],
                                    op=OP.add)
            nc.gpsimd.tensor_tensor(out=MXg[:], in0=MXg[:], in1=vld[:],
                                    op=OP.mult)
            nc.gpsimd.tensor_scalar(out=vld[:], in0=vld[:], scalar1=-1.0,
                                    scalar2=None, op0=OP.add)
            nc.gpsimd.tensor_tensor(out=MXg[:], in0=MXg[:], in1=vld[:],
                                    op=OP.add)
            s['MX'] = MXg

        def stage_tr(g):
            s = G[g]
            MXT = psumS.tile([32, 128], F32, name=f"mxt{g}", tag="mxt")
            nc.tensor.transpose(MXT[:], s['MX'][:], iden[:])
            MXTs = pool.tile([32, 128], F32, name=f"MXTs{g}", tag=f"MXTs{g}")
            nc.scalar.copy(out=MXTs[:], in_=MXT[:])
            flat1 = dpool.tile([1, 4096], F32, name=f"flat1_{g}",
                               tag=f"flat1_{g}")
            in_v = MXTs[:].rearrange("s (h16 p) -> s h16 p", h16=8, p=16)
            out_v = flat1[:].rearrange("a (p s h16) -> (a s) h16 p",
                                       p=16, s=32, h16=8)
            qe = nc.sync if g % 2 == 0 else nc.scalar
            qe.dma_start(out=out_v, in_=in_v)
            SG_IN = pool.tile([16, 256], F32, name=f"SG_IN{g}", tag=f"SG_IN{g}")
            f1r = flat1[:].rearrange("a (p f) -> (a p) f", p=16)
            qe.dma_start(out=SG_IN[:], in_=f1r)
            s['SG_IN'] = SG_IN

        def stage_sg(g):
            s = G[g]
            # SGSV[:, 0:48] = compacted values, [:, 48:96] = validity
            SGSV = pool.tile([16, 96], F32, name=f"SGSV{g}", tag=f"SGSV{g}")
            NF = pool.tile([1, 2], U32, name=f"NF{g}", tag=f"NF{g}")
            for tl in range(TPG):
                nc.gpsimd.sparse_gather(
                    SGSV[:, tl * 24:(tl + 1) * 24],
                    s['SG_IN'][:, tl * 128:(tl + 1) * 128],
                    num_found=NF[:, tl:tl + 1])
            NF16 = pool.tile([16, 2], U32, name=f"NF16{g}", tag=f"NF16{g}")
            nc.gpsimd.partition_broadcast(NF16[:], NF[:], channels=16)
            NF16f = pool.tile([16, 2], F32, name=f"NF16f{g}", tag=f"NF16f{g}")
            nc.gpsimd.tensor_copy(NF16f[:], NF16[:])
            sv = SGSV[:, 48:96]
            nf_b = NF16f[:].rearrange("p t -> p t ()").broadcast_to([16, 2, 24])
            nc.gpsimd.tensor_tensor(
                out=sv.rearrange("p (t f) -> p t f", f=24),
                in0=SPc[:].rearrange("p (t f) -> p t f", f=24),
                in1=nf_b, op=OP.subtract)
            nc.gpsimd.tensor_scalar(out=sv, in0=sv, scalar1=0.0,
                                    scalar2=None, op0=OP.is_lt)
            sg = SGSV[:, 0:48]
            nc.gpsimd.tensor_tensor(out=sg, in0=sg, in1=sv, op=OP.mult)
            svm1 = pool.tile([16, 48], F32, name=f"svm1{g}", tag=f"svm1{g}")
            nc.gpsimd.tensor_scalar(out=svm1[:], in0=sv, scalar1=-1.0,
                                    scalar2=None, op0=OP.add)
            nc.gpsimd.tensor_tensor(out=sg, in0=sg, in1=svm1[:], op=OP.add)
            nc.gpsimd.tensor_scalar(out=sg, in0=sg, scalar1=0.0,
                                    scalar2=None, op0=OP.max)
            # hw line idx (flat>>5) as f32 for PE replication
            LI = pool.tile([16, 48], I32, name=f"LI{g}", tag=f"LI{g}")
            nc.vector.tensor_copy(LI[:], sg)
            nc.vector.tensor_scalar(out=LI[:], in0=LI[:], scalar1=5,
                                    scalar2=None, op0=OP.arith_shift_right)
            HWf = pool.tile([16, 48], F32, name=f"HWf{g}", tag=f"HWf{g}")
            nc.vector.tensor_copy(HWf[:], LI[:])
            s['SGSV'] = SGSV
            s['HWf'] = HWf

        def stage_b2(g):
            s = G[g]
            # merged bounce: one write [16,96], one read [128,12]
            flat2 = dpool.tile([1, 1536], F32, name=f"flat2_{g}",
                               tag=f"flat2_{g}")
            # DRAM layout (x, tl, f24, p2): slot s of col (tl,j) at linear
            # x*768 + tl*384 + j*128 + s
            f2w = flat2[:].rearrange("a (x t f p) -> (a p) x t f",
                                     p=16, x=2, t=2, f=24)
            qe = nc.sync if g % 2 == 1 else nc.scalar
            qe.dma_start(out=f2w, in_=s['SGSV'][:].rearrange(
                "p (x t f) -> p x t f", x=2, t=2, f=24))
            LTVT = pool.tile([128, 12], F32, name=f"LTVT{g}", tag=f"LTVT{g}")
            f2r = flat2[:].rearrange("a (x t j p) -> (a p) (x t j)",
                                     x=2, t=2, j=3, p=128)
            qe.dma_start(out=LTVT[:], in_=f2r)
            s['LT'] = LTVT[:, 0:6]
            s['VT'] = LTVT[:, 6:12]
            # idx replication via PE: psum[p, f] = sum_k REP[k,p]*HWf[k,f]
            psI = psumS.tile([128, 48], F32, name=f"psI{g}", tag="psI")
            nc.tensor.matmul(psI[:], lhsT=REP[:], rhs=s['HWf'][:],
                             start=True, stop=True, skip_group_check=True)
            IDXr = pool.tile([128, 48], I16, name=f"IDXr{g}", tag=f"IDXr{g}")
            nc.vector.tensor_copy(IDXr[:], psI[:])
            s['IDXr'] = IDXr

        def stage_gather(g, gsem):
            s = G[g]
            GV4 = pool.tile([128, 768], F16, name=f"GV4{g}", tag=f"GV4{g}")
            gcall = nc.gpsimd.dma_gather(
                out_ap=GV4[:].rearrange("p (c k) -> p c k", k=128),
                in_ap=vals4_d[:].rearrange("p (w k) -> (p w) k", k=128),
                idxs_ap=s['IDXr'][:],
                num_idxs=768, num_idxs_reg=768, elem_size=128)
            gcall.then_inc(gsem, 16)
            s['GV4'] = GV4

        def stage_decode(g, gsem):
            s = G[g]
            E = t_eng(g)
            LTi = pool.tile([128, 6], I32, name=f"LTi{g}", tag=f"LTi{g}")
            nc.vector.tensor_copy(LTi[:], s['LT'])
            phi = pool.tile([128, 6], I32, name=f"phi{g}", tag=f"phi{g}")
            nc.vector.tensor_scalar(out=phi[:], in0=LTi[:], scalar1=13,
                                    scalar2=None, op0=OP.arith_shift_right)
            pwi = pool.tile([128, 6], I32, name=f"pwi{g}", tag=f"pwi{g}")
            nc.vector.tensor_scalar(out=pwi[:], in0=LTi[:], scalar1=6,
                                    scalar2=127, op0=OP.arith_shift_right,
                                    op1=OP.bitwise_and)
            pdi = pool.tile([128, 6], I32, name=f"pdi{g}", tag=f"pdi{g}")
            nc.vector.tensor_scalar(out=pdi[:], in0=LTi[:], scalar1=63,
                                    scalar2=None, op0=OP.bitwise_and)
            pdli = pool.tile([128, 6], I32, name=f"pdli{g}", tag=f"pdli{g}")
            nc.vector.tensor_scalar(out=pdli[:], in0=pdi[:], scalar1=31,
                                    scalar2=None, op0=OP.bitwise_and)
            ph = pool.tile([128, 6], F32, name=f"ph{g}", tag=f"ph{g}")
            E.tensor_copy(ph[:], phi[:])
            pw = pool.tile([128, 6], F32, name=f"pw{g}", tag=f"pw{g}")
            E.tensor_copy(pw[:], pwi[:])
            pd = pool.tile([128, 6], F32, name=f"pd{g}", tag=f"pd{g}")
            E.tensor_copy(pd[:], pdi[:])
            pdlf = pool.tile([128, 6], F32, name=f"pdlf{g}", tag=f"pdlf{g}")
            E.tensor_copy(pdlf[:], pdli[:])
            pdh2 = pool.tile([128, 12], F16, name=f"pdh2{g}", tag=f"pdh2{g}")
            pd3 = pdlf[:].rearrange("p c -> p c ()").broadcast_to([128, 6, 2])
            E.tensor_copy(pdh2[:].rearrange("p (c u) -> p c u", u=2), pd3)
            OH = pool.tile([128, 192], F16, name=f"OH{g}", tag=f"OH{g}")
            io32_b = io32h[:].rearrange(
                "p (k2 u) -> p () k2 u", u=2).broadcast_to([128, 6, 16, 2])
            pdh_b = pdh2[:].rearrange(
                "p (c u) -> p c () u", u=2).broadcast_to([128, 6, 16, 2])
            E.tensor_tensor(
                out=OH[:].rearrange("p (c k2 u) -> p c k2 u", k2=16, u=2),
                in0=io32_b, in1=pdh_b, op=OP.subtract)
            E.tensor_scalar(out=OH[:], in0=OH[:], scalar1=0.0,
                            scalar2=None, op0=OP.is_equal)
            GVm = pool.tile([128, 768], F16, name=f"GVm{g}", tag=f"GVm{g}")
            oh_b = OH[:].rearrange(
                "p (c k) -> p c k ()", k=32).broadcast_to([128, 6, 32, 4])
            mm = E.tensor_tensor(
                out=GVm[:].rearrange("p (c s v) -> p c s v", s=32, v=4),
                in0=s['GV4'][:].rearrange("p (c s v) -> p c s v", s=32, v=4),
                in1=oh_b, op=OP.mult)
            mm._wait_ge(gsem, 16)
            gvm4 = GVm[:].rearrange("p (c s v) -> p c s v", s=32, v=4)
            n = 16
            while n >= 1:
                E.tensor_tensor(out=gvm4[:, :, 0:n, :], in0=gvm4[:, :, 0:n, :],
                                in1=gvm4[:, :, n:2 * n, :], op=OP.add)
                n //= 2
            V24 = pool.tile([128, 24], F32, name=f"V24{g}", tag=f"V24{g}")
            E.tensor_copy(V24[:].rearrange("p (c v) -> p c () v", v=4),
                          gvm4[:, :, 0:1, :])
            s['V24'] = V24
            s['ph'], s['pw'], s['pd'] = ph, pw, pd

        def stage_prof(g):
            s = G[g]
            E = t_eng(g)
            v4 = s['V24'][:].rearrange("p (c v) -> p c v", v=4)
            VX, VY, VZ, VI = v4[:, :, 0], v4[:, :, 1], v4[:, :, 2], v4[:, :, 3]
            # fused axis profiles: ARG[p, (a, c, tap)] (a: x, y, z)
            ARG = pool.tile([128, 162], F32, name=f"ARG{g}", tag=f"ARG{g}")
            va = s['V24'][:].rearrange("p (c v) -> p v c", v=4)[:, 0:3, :]
            va_b = va.rearrange("p a c -> p a c ()").broadcast_to(
                [128, 3, 6, 9])
            io9_b3 = io9[:].rearrange("p t -> p () () t").broadcast_to(
                [128, 3, 6, 9])
            arg3 = ARG[:].rearrange("p (a c t) -> p a c t", a=3, t=9)
            E.tensor_tensor(out=arg3, in0=io9_b3, in1=va_b, op=OP.subtract)
            E.tensor_scalar(out=ARG[:], in0=ARG[:], scalar1=SINV,
                            scalar2=None, op0=OP.mult)
            E.tensor_tensor(out=ARG[:], in0=ARG[:], in1=ARG[:], op=OP.mult)
            A3 = pool.tile([128, 162], F16, name=f"A3{g}", tag=f"A3{g}")
            nc.scalar.activation(A3[:], ARG[:], AF.Exp, scale=-1.0)
            # 9-tap-exact normalization
            S3 = pool.tile([128, 18], F32, name=f"S3{g}", tag=f"S3{g}")
            nc.vector.tensor_reduce(
                out=S3[:], axis=X,
                in_=A3[:].rearrange("p (ac t) -> p ac t", t=9), op=OP.add)
            s3v = S3[:].rearrange("p (a c) -> p a c", a=3)
            amp = pool.tile([128, 6], F32, name=f"amp{g}", tag=f"amp{g}")
            E.tensor_tensor(out=amp[:], in0=s3v[:, 0, :], in1=s3v[:, 1, :],
                            op=OP.mult)
            E.tensor_tensor(out=amp[:], in0=amp[:], in1=s3v[:, 2, :],
                            op=OP.mult)
            nc.vector.reciprocal(amp[:], amp[:])
            E.tensor_tensor(out=amp[:], in0=amp[:], in1=VI, op=OP.mult)
            sc_b = scb_t[:].broadcast_to([128, 6])
            E.tensor_tensor(out=amp[:], in0=amp[:], in1=sc_b, op=OP.mult)
            E.tensor_tensor(out=amp[:], in0=amp[:], in1=s['VT'], op=OP.mult)
            # LH: amp-scaled compact X rows placed via local_scatter
            D9 = pool.tile([128, 54], F16, name=f"D9{g}", tag=f"D9{g}")
            amp_b = amp[:].rearrange("p c -> p c ()").broadcast_to([128, 6, 9])
            E.tensor_tensor(out=D9[:].rearrange("p (c t) -> p c t", t=9),
                            in0=A3[:, 0:54].rearrange("p (c t) -> p c t", t=9),
                            in1=amp_b, op=OP.mult)
            posl = pool.tile([128, 54], F32, name=f"posl{g}", tag=f"posl{g}")
            ph_b = s['ph'][:].rearrange("p c -> p c ()").broadcast_to(
                [128, 6, 9])
            io9_b = io9[:].rearrange("p t -> p () t").broadcast_to([128, 6, 9])
            E.tensor_tensor(out=posl[:].rearrange("p (c t) -> p c t", t=9),
                            in0=io9_b, in1=ph_b, op=OP.add)
            mbad = pool.tile([128, 54], F32, name=f"mbad{g}", tag=f"mbad{g}")
            E.tensor_scalar(out=mbad[:], in0=posl[:], scalar1=0.0,
                            scalar2=None, op0=OP.is_lt)
            mb2 = pool.tile([128, 54], F32, name=f"mb2{g}", tag=f"mb2{g}")
            E.tensor_scalar(out=mb2[:], in0=posl[:], scalar1=127.0,
                            scalar2=None, op0=OP.is_gt)
            E.tensor_tensor(out=mbad[:], in0=mbad[:], in1=mb2[:], op=OP.add)
            E.tensor_tensor(out=posl[:], in0=posl[:], in1=ib54[:], op=OP.add)
            E.tensor_scalar(out=mbad[:], in0=mbad[:], scalar1=-9999.0,
                            scalar2=None, op0=OP.mult)
            E.tensor_tensor(out=posl[:], in0=posl[:], in1=mbad[:], op=OP.add)
            I9 = pool.tile([128, 54], I16, name=f"I9{g}", tag=f"I9{g}")
            E.tensor_copy(I9[:], posl[:])
            LH = pool.tile([128, 768], F16, name=f"LH{g}", tag=f"LH{g}")
            nc.gpsimd.local_scatter(out_ap=LH[:], data_ap=D9[:], idxs_ap=I9[:],
                                    channels=128, num_elems=768, num_idxs=54)
            s['LH'] = LH
            # LY (dup layout) / LZ
            poY = pool.tile([128, 6], F32, name=f"poY{g}", tag=f"poY{g}")
            E.tensor_tensor(out=poY[:], in0=s['pw'][:], in1=woff[:, g * 6:
                            g * 6 + 6], op=OP.subtract)
            E.tensor_tensor(out=poY[:], in0=poY[:], in1=VY, op=OP.add)
            argY = pool.tile([128, 144], F32, name=f"argY{g}", tag=f"argY{g}")
            io24_b = io24[:].rearrange("p j -> p () j").broadcast_to(
                [128, 6, 24])
            poY_b = poY[:].rearrange("p c -> p c ()").broadcast_to([128, 6, 24])
            E.tensor_tensor(out=argY[:].rearrange("p (c j) -> p c j", j=24),
                            in0=io24_b, in1=poY_b, op=OP.subtract)
            E.tensor_scalar(out=argY[:], in0=argY[:], scalar1=SINV,
                            scalar2=None, op0=OP.mult)
            E.tensor_tensor(out=argY[:], in0=argY[:], in1=argY[:], op=OP.mult)
            LY = pool.tile([128, 144], F16, name=f"LY{g}", tag=f"LY{g}")
            nc.scalar.activation(LY[:], argY[:], AF.Exp, scale=-1.0)
            LY2 = pool.tile([128, 288], F16, name=f"LY2{g}", tag=f"LY2{g}")
            ly_b = LY[:].rearrange("p (c j) -> p c j ()", j=24).broadcast_to(
                [128, 6, 24, 2])
            E.tensor_copy(LY2[:].rearrange("p (c j u) -> p c j u", j=24, u=2),
                          ly_b)
            poZ = pool.tile([128, 6], F32, name=f"poZ{g}", tag=f"poZ{g}")
            E.tensor_tensor(out=poZ[:], in0=s['pd'][:], in1=VZ, op=OP.add)
            argZ = pool.tile([128, 384], F32, name=f"argZ{g}", tag=f"argZ{g}")
            io64_b = io64[:].rearrange("p k -> p () k").broadcast_to(
                [128, 6, 64])
            poZ_b = poZ[:].rearrange("p c -> p c ()").broadcast_to([128, 6, 64])
            E.tensor_tensor(out=argZ[:].rearrange("p (c k) -> p c k", k=64),
                            in0=io64_b, in1=poZ_b, op=OP.subtract)
            E.tensor_scalar(out=argZ[:], in0=argZ[:], scalar1=SINV,
                            scalar2=None, op0=OP.mult)
            E.tensor_tensor(out=argZ[:], in0=argZ[:], in1=argZ[:], op=OP.mult)
            LZ = pool.tile([128, 384], F16, name=f"LZ{g}", tag=f"LZ{g}")
            nc.scalar.activation(LZ[:], argZ[:], AF.Exp, scale=-1.0)
            s['LY2'] = LY2
            s['LZ'] = LZ

        def stage_rhs(g):
            s = G[g]
            RHS = pool.tile([128, CPG * WJ * WK], F16, name=f"RHS{g}",
                            tag=f"RHS{g}")
            n_pool = 2 if g < 2 else 3
            for i in range(CPG):
                lz_b = s['LZ'][:, i * WK:(i + 1) * WK].rearrange(
                    "p k -> p () k").broadcast_to([128, WJ, WK])
                if i >= CPG - n_pool:
                    ly_b = s['LY2'][:, i * 48:(i + 1) * 48].rearrange(
                        "p (j u) -> p j u", u=2)[:, :, 0:1].broadcast_to(
                        [128, WJ, WK])
                    nc.gpsimd.tensor_tensor(
                        out=RHS[:, i * WJ * WK:(i + 1) * WJ * WK].rearrange(
                            "p (j k) -> p j k", k=WK),
                        in0=ly_b, in1=lz_b, op=OP.mult)
                else:
                    ly_d = s['LY2'][:, i * 48:(i + 1) * 48].rearrange(
                        "p (j u) -> p j () u", u=2).broadcast_to(
                        [128, WJ, 32, 2])
                    rhs4 = RHS[:, i * WJ * WK:(i + 1) * WJ * WK].rearrange(
                        "p (j k2 u) -> p j k2 u", k2=32, u=2)
                    lz_d = s['LZ'][:, i * WK:(i + 1) * WK].rearrange(
                        "p (k2 u) -> p () k2 u", u=2).broadcast_to(
                        [128, WJ, 32, 2])
                    nc.vector.tensor_tensor(out=rhs4, in0=ly_d, in1=lz_d,
                                            op=OP.mult)
            s['RHS'] = RHS

        def warmup(n):
            psW = psumS.tile([128, 512], F32, name="psW", tag="psW")
            for _ in range(n):
                nc.tensor.matmul(psW[:], lhsT=wrm[:, 0:128], rhs=wrm[:],
                                 start=True, stop=True, skip_group_check=True)

        def stage_mm(b):
            # psum for bucket b: w in [16b, 16b+16) x 64 d, 2 banks.
            ps = psumP.tile([128, 1024], F32, name=f"ps{b}", tag="ps")
            for bank in range(2):
                mms = [(b, j, 256 + 512 * bank, 512, 512 * bank)
                       for j in range(NBPB)]
                if bank == 0 and b > 0:
                    mms += [(b - 1, j, 1280, 256, 0) for j in range(NBPB)]
                if bank == 1 and b < NBUCK - 1:
                    mms += [(b + 1, j, 0, 256, 768) for j in range(NBPB)]
                for i, (t, j, c0, wd, p0) in enumerate(mms):
                    gt, lc = t // 2, (t % 2) * 3 + j
                    nc.tensor.matmul(
                        ps[:, p0:p0 + wd],
                        lhsT=G[gt]['LH'][:, lc * 128:(lc + 1) * 128],
                        rhs=G[gt]['RHS'][:, lc * WJ * WK + c0:
                                         lc * WJ * WK + c0 + wd],
                        start=(i == 0), stop=(i == len(mms) - 1),
                        skip_group_check=True)
            ot = outp.tile([128, 1024], F16, name=f"ot{b}", tag="ot")
            ce = COPY_ENG[b]
            if ce == 'act':
                nc.scalar.copy(out=ot[:], in_=ps[:])
            elif ce == 'dve':
                nc.vector.tensor_copy(ot[:], ps[:])
            else:
                nc.gpsimd.tensor_copy(ot[:], ps[:])
            qe = nc.sync if b % 2 == 0 else nc.scalar
            qe.dma_start(out=out_d[:, b * 1024:(b + 1) * 1024], in_=ot[:])

        # ---------------- staggered issue ----------------
        gsems = [nc.alloc_semaphore(f"gsem{i}_{nc.next_id()}")
                 for i in range(NG)]
        for g in range(NG):
            stage_load(g)
        stage_max(0)
        stage_max(1)
        stage_mxpost(0); stage_tr(0); stage_sg(0); stage_b2(0)
        stage_gather(0, gsems[0])
        stage_max(2)
        stage_mxpost(1); stage_tr(1); stage_sg(1); stage_b2(1)
        stage_gather(1, gsems[1])
        stage_max(3)
        stage_mxpost(2); stage_tr(2); stage_sg(2); stage_b2(2)
        stage_gather(2, gsems[2])
        stage_decode(0, gsems[0]); stage_prof(0); stage_rhs(0)
        stage_mxpost(3); stage_tr(3); stage_sg(3); stage_b2(3)
        stage_gather(3, gsems[3])
        stage_decode(1, gsems[1]); stage_prof(1); stage_rhs(1)
        warmup(NWARM)
        stage_mm(0)
        stage_decode(2, gsems[2]); stage_prof(2); stage_rhs(2)
        stage_mm(1); stage_mm(2)
        stage_decode(3, gsems[3]); stage_prof(3); stage_rhs(3)
        stage_mm(3); stage_mm(4)
        stage_mm(5); stage_mm(6); stage_mm(7)


def build_nc(repeats=1, debug=False):
    nc = bacc.Bacc("TRN2", target_bir_lowering=False, debug=False,
                   num_devices=N_CORES)
    ins = []
    for nm in IN_NAMES:
        if nm == "locs":
            t = nc.dram_tensor(nm, [128, 8192], F16, kind="ExternalInput")
        elif nm == "vals4":
            t = nc.dram_tensor(nm, [128, 32768], F16, kind="ExternalInput")
        else:
            t = nc.dram_tensor(nm, [128, 1], F32, kind="ExternalInput")
        ins.append(t.ap())
    out_d = nc.dram_tensor("out", [128, W * D], F16, kind="ExternalOutput").ap()
    with tile.TileContext(nc) as tc:
        for _rep in range(repeats):
            body_v3(tc, [out_d], ins)
    nc.compile()
    return nc


class _SpmdRunner:
    def __init__(self, nc, n_cores=N_CORES):
        import jax
        import jax.numpy as jnp
        from jax.sharding import Mesh, PartitionSpec
        from jax.experimental.shard_map import shard_map
        from concourse import bass2jax
        from concourse.bass2jax import _bass_exec_p, partition_id_tensor
        bass2jax.install_neuronx_cc_hook()
        self.jax, self.jnp = jax, jnp
        self.n_cores = n_cores
        in_names, out_names, out_avals, zero_outs = [], [], [], []
        pname = nc.partition_id_tensor.name if nc.partition_id_tensor else None
        for alloc in nc.m.functions[0].allocations:
            if not isinstance(alloc, mybir.MemoryLocationSet):
                continue
            name = alloc.memorylocations[0].name
            if alloc.kind == "ExternalInput":
                if name != pname:
                    in_names.append(name)
            elif alloc.kind == "ExternalOutput":
                shape = tuple(alloc.tensor_shape)
                dtype = mybir.dt.np(alloc.dtype)
                out_names.append(name)
                out_avals.append(jax.core.ShapedArray(shape, dtype))
                zero_outs.append(np.zeros(shape, dtype))
        self.in_names, self.out_names = in_names, out_names
        self.out_avals, self.zero_outs = out_avals, zero_outs
        n_params, n_outs = len(in_names), len(out_avals)
        all_in = in_names + out_names + ([pname] if pname else [])

        def _fn(*args):
            operands = list(args)
            if pname is not None:
                operands.append(partition_id_tensor())
            return tuple(_bass_exec_p.bind(
                *operands, out_avals=tuple(out_avals), in_names=tuple(all_in),
                out_names=tuple(out_names), lowering_input_output_aliases=(),
                sim_require_finite=True, sim_require_nnan=True, nc=nc))

        devices = jax.devices()[:n_cores]
        mesh = Mesh(np.asarray(devices), ("core",))
        specs = (PartitionSpec("core"),)
        self.sharded = jax.jit(
            shard_map(_fn, mesh=mesh, in_specs=specs * (n_params + n_outs),
                      out_specs=specs * n_outs),
            donate_argnums=tuple(range(n_params, n_params + n_outs)),
            keep_unused=True)

    def run(self, in_maps):
        concat = [np.concatenate([np.asarray(m[n]) for m in in_maps], axis=0)
                  for n in self.in_names]
        zeros = [self.jnp.zeros((self.n_cores * z.shape[0], *z.shape[1:]),
                                z.dtype) for z in self.zero_outs]
        outs = self.sharded(*concat, *zeros)
        self.jax.block_until_ready(outs)
        return [
            {n: np.asarray(outs[i]).reshape(self.n_cores,
                                            *self.out_avals[i].shape)[c]
             for i, n in enumerate(self.out_names)}
            for c in range(self.n_cores)]


_RUNNER_CACHE = {}


def _get_runner(repeats=1):
    if repeats not in _RUNNER_CACHE:
        _RUNNER_CACHE[repeats] = _SpmdRunner(build_nc(repeats))
    return _RUNNER_CACHE[repeats]


def _make_in_maps(locs_3d, x_os_3d, y_os_3d, z_os_3d, ints_3d, scale):
    sc = float(np.asarray(scale).reshape(-1)[0])
    scb = np.full((128, 1), 1000.0 * sc, np.float32)
    iota512 = np.tile(np.arange(512, dtype=np.float32) + 1.0, 16)
    in_maps = []
    for c in range(N_CORES):
        locs = np.asarray(locs_3d)[c, 0].reshape(128, 8192)
        locs_pre = (locs * iota512).astype(np.float16)
        sl = [np.asarray(t)[c, 0].reshape(128, 128, 64)
              for t in (x_os_3d, y_os_3d, z_os_3d, ints_3d)]
        vals4 = np.stack(sl, axis=3).astype(np.float16).reshape(128, 32768)
        in_maps.append({"locs": locs_pre, "vals4": np.ascontiguousarray(vals4),
                        "scb": scb})
    return in_maps


def kernel(locs_3d, x_os_3d, y_os_3d, z_os_3d, ints_3d, scale):
    runner = _get_runner()
    in_maps = _make_in_maps(locs_3d, x_os_3d, y_os_3d, z_os_3d, ints_3d, scale)
    res = runner.run(in_maps)
    out = np.stack([res[c]["out"].astype(np.float32).reshape(H, W, D)
                    for c in range(N_CORES)])
    return out[:, None]
